# revision 13
# baseline (speedup 1.0000x reference)
"""Multi-head attention (B=2, T=2048, D=1024, H=16, no causal mask) on 8 trn2
NeuronCores.

Sharding: pure data-parallel over (batch, query-token-block).  Core c handles
batch b = c // 4 and query rows [tb*512, (tb+1)*512) with tb = c % 4.  Each
core redundantly computes K and V for its whole batch (15.1 GFLOP/core vs 8.6
for tensor-parallel-heads) but needs NO collectives; an on-chip 4-rank 8 MB
AllReduce would cost more than the redundant compute.

Per-core plan (all matmuls in float32r -> full PE rate at N=512):
  1. PE-transpose X[b] (2048x1024) into XT (1024x2048 on SBUF) and the core's
     query slice Xq into XqT.
  2. QT[do,:]  = Wq[:,do]^T @ XqT      (QT:  [1024, 512]  SBUF, persistent)
     KT[do,:]  = Wk[:,do]^T @ XT       (KT:  [1024, 2048] spilled to DRAM)
     V [tc,:]  = XT[:,tc]^T @ Wv       (V:   [2048, 1024] SBUF, 65-wide head
                                        slots with a ones column appended ->
                                        PV matmul also produces the softmax
                                        denominator for free)
  3. Attention per head pair p (heads 2p, 2p+1 live at partitions 0-63 /
     64-127 of KT/QT row chunk p -> row-tiled concurrent K=64 matmuls):
       logitsT[k,q] = KT_h[:,kc]^T @ QT_h      (PSUM [128, 512])
       PT = exp(0.125 * logitsT)               (ScalarE, PSUM -> SBUF)
       outT_h[dh,q], s[q] = [V_h | 1]^T @ PT   (PSUM [65, 512], accum 16 kc)
     normalize: outT_h *= (1/s) broadcast across partitions via DMA.
  4. y[q,:] = outT^T @ Wo  (accumulate 8 row chunks), DMA PSUM -> DRAM.
"""

import numpy as np

import concourse.bass as bass
import concourse.bacc as bacc
import concourse.mybir as mybir
import concourse.tile as tile
from concourse.masks import make_identity

F32 = mybir.dt.float32
F32R = mybir.dt.float32r

B, T, D, H = 2, 2048, 1024, 16
DH = D // H  # 64
TQ = 512     # query tokens per core
N_CORES = 8
P = 128
KD = D // P        # 8 contraction chunks over D
NT = T // P        # 16 key-token chunks
NTB = T // TQ      # 4 token blocks
NPAIR = H // 2     # 8 head pairs
VW = DH + 1        # 65: V head slot width incl. ones column
NQ = TQ // P       # 4 query-token chunks
EXPF = mybir.ActivationFunctionType.Exp

# float32r = fp32 data on the fast (1 cycle/row at N>=256) PE path.
MM_DT = F32R


def _m(ap):
    return ap


def build_nc():
    nc = bacc.Bacc("TRN2", target_bir_lowering=False, debug=False,
                   num_devices=N_CORES)
    xb = nc.dram_tensor("xb", [T, D], F32, kind="ExternalInput").ap()
    xq = nc.dram_tensor("xq", [TQ, D], F32, kind="ExternalInput").ap()
    wqkv = nc.dram_tensor("wqkv", [D, 3 * D], F32R, kind="ExternalInput").ap()
    wo = nc.dram_tensor("wo", [D, D], F32R, kind="ExternalInput").ap()
    y = nc.dram_tensor("y", [TQ, D], F32, kind="ExternalOutput").ap()
    ktd = nc.dram_tensor("kt_scratch", [D, T], F32R).ap()

    with tile.TileContext(nc) as tc:
        with tc.tile_pool(name="persist", bufs=1) as persist:
            v_sb = persist.tile([P, NT * H * VW], F32R)     # 65 KB/part
            qt_sb = persist.tile([P, NPAIR * TQ], F32R)     # 16 KB/part
            ident = persist.tile([P, P], F32)
            make_identity(nc, ident)
            # ones columns in every (tok-chunk, head) V slot.  memset on an
            # f32r view is invalid ISA, so memset an f32 column and copy
            # (the copy rounds to f32r).
            onec = persist.tile([P, 1], F32)
            nc.vector.memset(onec[:], 1.0)
            nc.vector.tensor_copy(
                v_sb.rearrange("p (b c) -> p b c", c=VW)[:, :, DH:DH + 1],
                onec.unsqueeze(1).broadcast_to((P, NT * H, 1)))

            # ---------------- phase A-D: transposes + projections ----------
            with (
                tc.tile_pool(name="xtp", bufs=1) as xtp,
                tc.tile_pool(name="xin", bufs=2) as xinp,
                tc.tile_pool(name="wp", bufs=1) as wp,
                tc.tile_pool(name="trps", bufs=4, space="PSUM") as trps,
                tc.tile_pool(name="pjps", bufs=4, space="PSUM") as pjps,
            ):
                xt = xtp.tile([P, KD * T], F32R)    # 64 KB/part
                xqt = xtp.tile([P, KD * TQ], F32R)  # 16 KB/part

                # A: transpose xq then xb (xq first so QT can finish early)
                for tci in range(NQ):
                    xin = xinp.tile([P, D], F32, tag="xin")
                    nc.sync.dma_start(xin[:], xq[tci * P:(tci + 1) * P, :])
                    for kd in range(KD):
                        ps = trps.tile([P, P], F32, tag="tr")
                        nc.tensor.transpose(
                            ps[:], xin[:, kd * P:(kd + 1) * P], ident[:])
                        nc.any.tensor_copy(
                            xqt[:, kd * TQ + tci * P: kd * TQ + (tci + 1) * P],
                            ps[:])
                for tci in range(NT):
                    xin = xinp.tile([P, D], F32, tag="xin")
                    nc.sync.dma_start(xin[:], xb[tci * P:(tci + 1) * P, :])
                    for kd in range(KD):
                        ps = trps.tile([P, P], F32, tag="tr")
                        nc.tensor.transpose(
                            ps[:], xin[:, kd * P:(kd + 1) * P], ident[:])
                        nc.any.tensor_copy(
                            xt[:, kd * T + tci * P: kd * T + (tci + 1) * P],
                            ps[:])

                # wqkv viewed as [p, ko, col] so a whole K-column strip of a
                # weight loads with one DMA
                wq3 = wqkv.rearrange("(ko p) c -> p ko c", p=P)

                # D: QT (dout chunk do covers heads 2do, 2do+1)
                for do in range(KD):
                    wt = wp.tile([P, KD * P], F32R, tag="wk", bufs=2)
                    nc.sync.dma_start(
                        wt.rearrange("p (ko c) -> p ko c", c=P),
                        wq3[:, :, do * P:(do + 1) * P])
                    pq = pjps.tile([P, TQ], F32, tag="pj")
                    for kd in range(KD):
                        nc.tensor.matmul(
                            pq[:], _m(wt[:, kd * P:(kd + 1) * P]),
                            _m(xqt[:, kd * TQ:(kd + 1) * TQ]),
                            start=(kd == 0), stop=(kd == KD - 1))
                    nc.any.tensor_copy(qt_sb[:, do * TQ:(do + 1) * TQ], pq[:])

                # B: KT -> DRAM spill
                for do in range(KD):
                    wt = wp.tile([P, KD * P], F32R, tag="wk", bufs=2)
                    nc.sync.dma_start(
                        wt.rearrange("p (ko c) -> p ko c", c=P),
                        wq3[:, :, D + do * P: D + (do + 1) * P])
                    for tb in range(NTB):
                        pk = pjps.tile([P, TQ], F32, tag="pj")
                        for kd in range(KD):
                            nc.tensor.matmul(
                                pk[:], _m(wt[:, kd * P:(kd + 1) * P]),
                                _m(xt[:, kd * T + tb * TQ: kd * T + (tb + 1) * TQ]),
                                start=(kd == 0), stop=(kd == KD - 1))
                        ks = xinp.tile([P, TQ], F32R, tag="ktst", bufs=3)
                        nc.any.tensor_copy(ks[:], pk[:])
                        nc.sync.dma_start(
                            ktd[do * P:(do + 1) * P, tb * TQ:(tb + 1) * TQ],
                            ks[:])

                # C: V natural, into 65-wide head slots
                for nh in range(2):
                    wvt = wp.tile([P, KD * TQ], F32R, tag="wv", bufs=1)
                    nc.sync.dma_start(
                        wvt.rearrange("p (ko c) -> p ko c", c=TQ),
                        wq3[:, :, 2 * D + nh * TQ: 2 * D + (nh + 1) * TQ])
                    wv = [wvt[:, _i * TQ:(_i + 1) * TQ] for _i in range(KD)]
                    for tci in range(NT):
                        pv = pjps.tile([P, TQ], F32, tag="pj")
                        for kd in range(KD):
                            nc.tensor.matmul(
                                pv[:],
                                _m(xt[:, kd * T + tci * P: kd * T + (tci + 1) * P]),
                                _m(wv[kd][:]),
                                start=(kd == 0), stop=(kd == KD - 1))
                        dst = v_sb[:, tci * (H * VW) + nh * 8 * VW:
                                   tci * (H * VW) + (nh + 1) * 8 * VW]
                        nc.any.tensor_copy(
                            dst.rearrange("p (h c) -> p h c", c=VW)[:, :, 0:DH],
                            pv.rearrange("p (h c) -> p h c", c=DH))

            # ---------------- phase E: attention + F: output proj ----------
            with (
                tc.tile_pool(name="otp", bufs=1) as otp,
                tc.tile_pool(name="ktp", bufs=2) as ktp,
                tc.tile_pool(name="ptp", bufs=4) as ptp,
                tc.tile_pool(name="rcp", bufs=2) as rcp,
                tc.tile_pool(name="rbp", bufs=3) as rbp,
                tc.tile_pool(name="wop", bufs=16) as wop,
                tc.tile_pool(name="lgps", bufs=3, space="PSUM") as lgps,
                tc.tile_pool(name="pvps", bufs=3, space="PSUM") as pvps,
                tc.tile_pool(name="fps", bufs=2, space="PSUM") as fps,
            ):
                ot_sb = otp.tile([P, NPAIR * TQ], F32R)     # 16 KB/part
                for p in range(NPAIR):
                    kt = ktp.tile([P, T], F32R, tag="kt")
                    nc.sync.dma_start(kt[:], ktd[p * P:(p + 1) * P, :])
                    qa = qt_sb[0:DH, p * TQ:(p + 1) * TQ]
                    qb = qt_sb[DH:P, p * TQ:(p + 1) * TQ]
                    pva = pvps.tile([VW, TQ], F32, tag="pv")
                    pvb = pvps.tile([VW, TQ], F32, tag="pv")
                    pend = []  # software pipeline: PV of kc issued after logits kc+1
                    for kc in range(NT):
                        lga = lgps.tile([P, TQ], F32, tag="lg")
                        lgb = lgps.tile([P, TQ], F32, tag="lg")
                        nc.tensor.matmul(lga[:], _m(kt[0:DH, kc * P:(kc + 1) * P]),
                                         _m(qa), start=True, stop=True)
                        nc.tensor.matmul(lgb[:], _m(kt[DH:P, kc * P:(kc + 1) * P]),
                                         _m(qb), start=True, stop=True)
                        pta = ptp.tile([P, TQ], F32R, tag="pt")
                        ptb = ptp.tile([P, TQ], F32R, tag="pt")
                        nc.scalar.activation(pta[:], lga[:], EXPF, scale=0.125)
                        nc.scalar.activation(ptb[:], lgb[:], EXPF, scale=0.125)
                        for (pt_, pv_, h) in pend:
                            kcp = pend_kc
                            va = v_sb[:, kcp * (H * VW) + h * VW:
                                      kcp * (H * VW) + h * VW + VW]
                            nc.tensor.matmul(pv_[:], _m(va), _m(pt_[:]),
                                             start=(kcp == 0),
                                             stop=(kcp == NT - 1))
                        pend = [(pta, pva, 2 * p), (ptb, pvb, 2 * p + 1)]
                        pend_kc = kc
                    for (pt_, pv_, h) in pend:
                        va = v_sb[:, pend_kc * (H * VW) + h * VW:
                                  pend_kc * (H * VW) + h * VW + VW]
                        nc.tensor.matmul(pv_[:], _m(va), _m(pt_[:]),
                                         start=(pend_kc == 0), stop=True)

                    # normalize: outT_h[dh, q] *= 1 / s[q]
                    for hi, pv_ in ((0, pva), (1, pvb)):
                        rc = rcp.tile([P, TQ], F32, tag="rc")
                        nc.vector.reciprocal(rc[DH:DH + 1, :], pv_[DH:DH + 1, :])
                        rb = rbp.tile([P, TQ], F32, tag="rb")
                        nc.sync.dma_start(
                            rb[0:DH, :],
                            rc[DH:DH + 1, :].unsqueeze(1)
                              .broadcast_to((1, DH, TQ)))
                        if hi == 0:
                            nc.vector.tensor_mul(
                                ot_sb[0:DH, p * TQ:(p + 1) * TQ],
                                pv_[0:DH, :], rb[0:DH, :])
                        else:
                            # head b lands at partitions 64-127 of ot_sb, but
                            # DVE cannot shift partitions: normalize into a
                            # staging tile then DMA-shift partitions.
                            sh = rbp.tile([P, TQ], F32R, tag="sh")
                            nc.vector.tensor_mul(
                                sh[0:DH, :], pv_[0:DH, :], rb[0:DH, :])
                            nc.sync.dma_start(
                                ot_sb[DH:P, p * TQ:(p + 1) * TQ], sh[0:DH, :])

                # F: y = outT^T @ Wo
                wot = {}
                for ph in range(NPAIR):
                    for nh in range(2):
                        wot[ph, nh] = wop.tile([P, TQ], F32R, tag="wo",
                                               name=f"wo_{ph}_{nh}")
                        nc.sync.dma_start(
                            wot[ph, nh][:],
                            wo[ph * P:(ph + 1) * P, nh * TQ:(nh + 1) * TQ])
                for qc in range(NQ):
                    for nh in range(2):
                        py = fps.tile([P, TQ], F32, tag="f")
                        for ph in range(NPAIR):
                            nc.tensor.matmul(
                                py[:],
                                _m(ot_sb[:, ph * TQ + qc * P: ph * TQ + (qc + 1) * P]),
                                _m(wot[ph, nh][:]),
                                start=(ph == 0), stop=(ph == NPAIR - 1))
                        ys = rbp.tile([P, TQ], F32, tag="rb")
                        nc.any.tensor_copy(ys[:], py[:])
                        nc.sync.dma_start(
                            y[qc * P:(qc + 1) * P, nh * TQ:(nh + 1) * TQ],
                            ys[:])
    nc.compile()
    return nc


_NC_CACHE = None


def _get_nc():
    global _NC_CACHE
    if _NC_CACHE is None:
        _NC_CACHE = build_nc()
    return _NC_CACHE


def kernel(x, Wqkv, Wo):
    from concourse.bass_utils import run_bass_kernel_spmd

    x = np.ascontiguousarray(np.asarray(x, dtype=np.float32))
    Wqkv = np.ascontiguousarray(np.asarray(Wqkv, dtype=np.float32))
    Wo = np.ascontiguousarray(np.asarray(Wo, dtype=np.float32))
    nc = _get_nc()
    in_maps = []
    for c in range(N_CORES):
        b, tb = c // NTB, c % NTB
        in_maps.append({
            "xb": x[b],
            "xq": np.ascontiguousarray(x[b, tb * TQ:(tb + 1) * TQ, :]),
            "wqkv": Wqkv,
            "wo": Wo,
        })
    res = run_bass_kernel_spmd(nc, in_maps, core_ids=list(range(N_CORES)))
    out = np.empty((B, T, D), dtype=np.float32)
    for c in range(N_CORES):
        b, tb = c // NTB, c % NTB
        out[b, tb * TQ:(tb + 1) * TQ, :] = res.results[c]["y"]
    return out
